# revision 21
# baseline (speedup 1.0000x reference)
"""MultiHeadAttention Trainium2 Bass kernel.

Reference math (B=4, L=2048, D=512, H=8, HD=64):
    qh = relu(q @ Wq.T + bq) ; kh = relu(k @ Wk.T + bk) ; vh = relu(v @ Wv.T + bv)
    scores = (qh_heads @ kh_heads.T) / sqrt(512)
    out = softmax(scores) @ vh_heads   (per head)
    out = relu(concat_heads(out) @ Wo.T + bo)
rel_k_table / rel_v_table are dead inputs.

Sharding: 8 cores = (batch b in 0..3) x (query-half lh in 0..1).
Each core computes the full output rows [b, lh*1024:(lh+1)*1024, :].
k/v projections are duplicated between the 2 cores of a batch (cheap) so
there is NO cross-core communication; host only transposes/concats.

On-device layout is fully transposed ("feature dim on partitions"):
  inputs uploaded as qT [512,1024], kT [512,2048], vT [512,2048] (host transpose)
  qhT/khT: [64*H rows, L] computed as  W.T-tiles (lhsT) @ xT (rhs)   [f32r]
  vh:      natural [l, do] via xT-tiles (lhsT) @ WvT (rhs), bias via ones-row
           matmul, relu on DVE -> bf16, augmented with a ones column (65 wide)
  scores_T[k, q] per head = khT-tile (lhsT, K=dh=64) @ qhT (rhs)     [f32r]
  P = exp(scores * 1/sqrt(512)) on ACT (N=1024 per instr) -> bf16
  attnV natural: P_T-tile (lhsT) @ vh_aug (rhs, N=65) -> [q,64|denom] [bf16]
  normalize rows by 1/denom (per-partition scalar on DVE), PE-transpose
  final_T = WoT-tiles (lhsT) @ oVT (rhs) + relu/bias                 [f32r]
  output stored transposed [512, 1024]; host transposes back.
"""

import sys

sys.path.insert(0, "/opt/trn_rl_repo")

import numpy as np

import concourse.bass as bass
import concourse.mybir as mybir
from concourse.tile import TileContext
from concourse.bass_utils import run_bass_kernel_spmd

B, L, D, H = 4, 2048, 512, 8
HD = D // H  # 64
Q = L // 2  # queries per core (1024)
SCALE = 1.0 / float(np.sqrt(D))

F32 = mybir.dt.float32
F32R = mybir.dt.float32r
BF16 = mybir.dt.bfloat16

N_CORES = 8


def _split_sem_waits(nc, max_waits=1):
    """walrus in this container only accepts one sem-wait per instruction;
    split extra waits onto preceding NoOps on the same engine."""
    ctr = [0]

    def mknop(engine, waits):
        ctr[0] += 1
        n = mybir.InstNoOp(name=f"I-waitfix-{ctr[0]}", ins=[], outs=[])
        n.engine = engine
        n.sync_info = mybir.SyncInfo(on_wait=list(waits), on_update=[])
        return n

    for fn in nc.m.functions:
        for bb in fn.blocks:
            changed = False
            new = []
            for inst in bb.instructions:
                si = inst.sync_info
                if si is not None and si.on_wait and len(si.on_wait) > max_waits:
                    waits = list(si.on_wait)
                    extra, keep = waits[:-max_waits], waits[-max_waits:]
                    for i in range(0, len(extra), max_waits):
                        new.append(mknop(inst.engine, extra[i : i + max_waits]))
                    inst.sync_info = mybir.SyncInfo(
                        on_wait=keep, on_update=list(si.on_update)
                    )
                    changed = True
                new.append(inst)
            if changed:
                bb.instructions = new


def _raise_sbuf_limit():
    try:
        from concourse import tile_utils

        if getattr(tile_utils, "max_sbuf_usage", 0) < 206 * 1024:
            tile_utils.max_sbuf_usage = 206 * 1024
    except Exception:
        pass


def build_nc(waitfix=True):
    _raise_sbuf_limit()
    nc = bass.Bass()

    qT = nc.declare_dram_parameter("qT", [D, Q], F32R, isOutput=False)
    kT = nc.declare_dram_parameter("kT", [D, L], F32R, isOutput=False)
    vT = nc.declare_dram_parameter("vT", [D, L], F32R, isOutput=False)
    WqT = nc.declare_dram_parameter("WqT", [D, D], F32R, isOutput=False)
    WkT = nc.declare_dram_parameter("WkT", [D, D], F32R, isOutput=False)
    WvT = nc.declare_dram_parameter("WvT", [D, D], F32R, isOutput=False)
    WoT = nc.declare_dram_parameter("WoT", [D, D], F32R, isOutput=False)
    # biases pre-tiled on host: [128, 4] column t = bias slice for do-tile t
    bqt = nc.declare_dram_parameter("bqt", [128, 4], F32, isOutput=False)
    bkt = nc.declare_dram_parameter("bkt", [128, 4], F32, isOutput=False)
    bot = nc.declare_dram_parameter("bot", [128, 4], F32, isOutput=False)
    bvr = nc.declare_dram_parameter("bvr", [1, D + 128], F32R, isOutput=False)
    idn = nc.declare_dram_parameter("idn", [128, 128], F32, isOutput=False)
    outT = nc.declare_dram_parameter("outT", [D, Q], F32, isOutput=True)

    # dram views with row-tiles on partitions
    qT_t = qT.rearrange("(t p) l -> p t l", p=128)  # [128, 4, Q]
    kT_t = kT.rearrange("(t p) l -> p t l", p=128)  # [128, 4, L]
    vT_t = vT.rearrange("(t p) l -> p t l", p=128)
    WqT_t = WqT.rearrange("(t p) d -> p t d", p=128)  # [128, 4, 512]
    WkT_t = WkT.rearrange("(t p) d -> p t d", p=128)
    WvT_t = WvT.rearrange("(t p) d -> p t d", p=128)
    WoT_t = WoT.rearrange("(t p) d -> p t d", p=128)
    outT_t = outT.rearrange("(t p) l -> p t l", p=128)

    KT = L // 128  # 16 k-tiles
    QT = Q // 128  # 8 q-tiles

    with TileContext(nc) as tc:
        with (
            tc.tile_pool(name="persist", bufs=1) as persist,
            tc.tile_pool(name="weights", bufs=1) as wpool,
            tc.tile_pool(name="stream", bufs=2) as stream,
            tc.tile_pool(name="pslab", bufs=2) as pslab,
            tc.tile_pool(name="outp", bufs=2) as outp,
            tc.tile_pool(name="ps_big", bufs=2, space="PSUM") as ps_big,
            tc.tile_pool(name="ps_small", bufs=2, space="PSUM") as ps_small,
            tc.tile_pool(name="ps_proj", bufs=1, space="PSUM") as ps_proj,
        ):
            # ---- constants / weights ----
            w_q = wpool.tile([128, 4, D], F32R, tag="wq")
            w_k = wpool.tile([128, 4, D], F32R, tag="wk")
            w_v = wpool.tile([128, 4, D], F32R, tag="wv")
            w_o = persist.tile([128, 4, D], F32R, tag="wo")
            nc.sync.dma_start(out=w_k, in_=WkT_t)
            nc.sync.dma_start(out=w_q, in_=WqT_t)
            nc.sync.dma_start(out=w_v, in_=WvT_t)
            nc.sync.dma_start(out=w_o, in_=WoT_t)
            b_q = persist.tile([128, 4], F32, tag="bq")
            b_k = persist.tile([128, 4], F32, tag="bk")
            b_o = persist.tile([128, 4], F32, tag="bo")
            b_v = persist.tile([1, D + 128], F32R, tag="bv")
            nc.sync.dma_start(out=b_q, in_=bqt[:, :])
            nc.sync.dma_start(out=b_k, in_=bkt[:, :])
            nc.sync.dma_start(out=b_o, in_=bot[:, :])
            nc.sync.dma_start(out=b_v, in_=bvr[:, :])
            ones_row = b_v[0:1, D : D + 128]

            # ---- persistent activations ----
            # khT / qhT: 4 tiles of [128, L] fp32; partition = feature (2 heads/tile)
            khT = [persist.tile([128, L], BF16, tag=f"khT{t}", name=f"khT{t}") for t in range(4)]
            qhT = [persist.tile([128, Q], BF16, tag=f"qhT{t}", name=f"qhT{t}") for t in range(4)]
            # vh natural, bf16, augmented ones col: [128, kt, h, 65]
            vh = persist.tile([128, KT, H, HD + 1], BF16, tag="vh")
            nc.vector.memset(vh[:, :, :, HD : HD + 1], 1.0)
            # attention output transposed: 4 tiles [128, Q] fp32
            oVT = [persist.tile([128, Q], F32R, tag=f"oVT{t}", name=f"oVT{t}") for t in range(4)]

            # ---- projections ----
            # khT[dt][:, :] = relu(WkT_tiles.T @ kT + bk); stream kT in 4 chunks
            LC = 4  # l-chunks of 512 for k/v
            for lc in range(LC):
                kc = stream.tile([128, 4, 512], F32R, tag="chunk")
                nc.sync.dma_start(out=kc, in_=kT_t[:, :, lc * 512 : (lc + 1) * 512])
                for dt in range(4):
                    ps = ps_proj.tile([128, 512], F32, tag="proj")
                    for kt in range(4):
                        nc.tensor.matmul(
                            ps,
                            lhsT=w_k[:, kt, dt * 128 : (dt + 1) * 128],
                            rhs=kc[:, kt, :],
                            start=(kt == 0),
                            stop=(kt == 3),
                        )
                    nc.vector.tensor_scalar(
                        out=khT[dt][:, lc * 512 : (lc + 1) * 512],
                        in0=ps,
                        scalar1=b_k[:, dt : dt + 1],
                        scalar2=0.0,
                        op0=mybir.AluOpType.add,
                        op1=mybir.AluOpType.max,
                    )
                # vh natural for this chunk: 4 l-tiles of 128
                vc = stream.tile([128, 4, 512], F32R, tag="chunk")
                nc.sync.dma_start(out=vc, in_=vT_t[:, :, lc * 512 : (lc + 1) * 512])
                for lt4 in range(4):
                    lt = lc * 4 + lt4  # global l-tile (= k-tile index)
                    ps = ps_proj.tile([128, 512], F32, tag="proj")
                    for kt in range(4):
                        nc.tensor.matmul(
                            ps,
                            lhsT=vc[:, kt, lt4 * 128 : (lt4 + 1) * 128],
                            rhs=w_v[:, kt, :],
                            start=(kt == 0),
                            stop=False,
                        )
                    nc.tensor.matmul(
                        ps,
                        lhsT=ones_row,
                        rhs=b_v[0:1, 0:D],
                        start=False,
                        stop=True,
                    )
                    nc.vector.tensor_scalar(
                        out=vh[:, lt, :, 0:HD],
                        in0=ps.rearrange("p (h d) -> p h d", h=H),
                        scalar1=0.0,
                        scalar2=None,
                        op0=mybir.AluOpType.max,
                    )
            for lc in range(2):  # q chunks (Q=1024)
                qc = stream.tile([128, 4, 512], F32R, tag="chunk")
                nc.sync.dma_start(out=qc, in_=qT_t[:, :, lc * 512 : (lc + 1) * 512])
                for dt in range(4):
                    ps = ps_proj.tile([128, 512], F32, tag="proj")
                    for kt in range(4):
                        nc.tensor.matmul(
                            ps,
                            lhsT=w_q[:, kt, dt * 128 : (dt + 1) * 128],
                            rhs=qc[:, kt, :],
                            start=(kt == 0),
                            stop=(kt == 3),
                        )
                    nc.vector.tensor_scalar(
                        out=qhT[dt][:, lc * 512 : (lc + 1) * 512],
                        in0=ps,
                        scalar1=b_q[:, dt : dt + 1],
                        scalar2=0.0,
                        op0=mybir.AluOpType.add,
                        op1=mybir.AluOpType.max,
                    )

            # ---- attention per head ----
            # Normalization runs one (h,qch) iteration behind the attnV
            # matmul groups so the PE stream never waits on DVE.
            def emit_norm(pend):
                rc, oU, tht, tho, qsl = pend
                ps_b = ps_small.tile([HD, 512], F32, tag="bcast", bufs=1)
                nc.tensor.matmul(
                    ps_b,
                    lhsT=ones_row[0:1, 0:HD],
                    rhs=rc,
                    start=True,
                    stop=True,
                )
                recipB = outp.tile([HD, 512], F32, tag="recipB", bufs=1)
                nc.vector.tensor_copy(out=recipB, in_=ps_b)
                nc.vector.tensor_tensor(
                    out=oVT[tht][tho : tho + 64, qsl],
                    in0=oU,
                    in1=recipB,
                    op=mybir.AluOpType.mult,
                )

            pend = None
            for h in range(H):
                ht, ho = h // 2, (h % 2) * 64
                P = pslab.tile([128, KT, Q], BF16, tag="P")
                for kt in range(KT):
                    ps_s = ps_big.tile([128, Q], F32, tag="scores")
                    for qh2 in range(2):
                        nc.tensor.matmul(
                            ps_s[:, qh2 * 512 : (qh2 + 1) * 512],
                            lhsT=khT[ht][ho : ho + 64, kt * 128 : (kt + 1) * 128],
                            rhs=qhT[ht][ho : ho + 64, qh2 * 512 : (qh2 + 1) * 512],
                            start=True,
                            stop=True,
                        )
                    nc.scalar.activation(
                        out=P[:, kt, :],
                        in_=ps_s,
                        func=mybir.ActivationFunctionType.Exp,
                        scale=SCALE,
                    )
                for qch in range(2):
                    qsl = slice(qch * 512, (qch + 1) * 512)
                    ps_av = ps_small.tile([HD + 1, 512], F32, tag="attnv")
                    for kt in range(KT):
                        nc.tensor.matmul(
                            ps_av,
                            lhsT=vh[:, kt, h, :],
                            rhs=P[:, kt, qsl],
                            start=(kt == 0),
                            stop=(kt == KT - 1),
                        )
                    oU = outp.tile([HD, 512], F32, tag="oU")
                    nc.vector.tensor_copy(out=oU, in_=ps_av[0:HD, :])
                    rc = outp.tile([1, 512], F32R, tag="fout")
                    with nc.allow_low_precision(reason="f32r recip bcast"):
                        nc.vector.reciprocal(out=rc, in_=ps_av[HD : HD + 1, :])
                    if pend is not None:
                        emit_norm(pend)
                    pend = (rc, oU, ht, ho, qsl)
            emit_norm(pend)

            # ---- output projection ----
            for qc2 in range(2):
                for dt in range(4):
                    ps = ps_proj.tile([128, 512], F32, tag="proj")
                    for ktile in range(4):
                        nc.tensor.matmul(
                            ps,
                            lhsT=w_o[:, ktile, dt * 128 : (dt + 1) * 128],
                            rhs=oVT[ktile][:, qc2 * 512 : (qc2 + 1) * 512],
                            start=(ktile == 0),
                            stop=(ktile == 3),
                        )
                    fo = outp.tile([128, 512], F32, tag="fout")
                    nc.vector.tensor_scalar(
                        out=fo,
                        in0=ps,
                        scalar1=b_o[:, dt : dt + 1],
                        scalar2=0.0,
                        op0=mybir.AluOpType.add,
                        op1=mybir.AluOpType.max,
                    )
                    nc.sync.dma_start(
                        out=outT_t[:, dt, qc2 * 512 : (qc2 + 1) * 512], in_=fo
                    )

    if waitfix:
        _split_sem_waits(nc, max_waits=1)
    return nc


_NC = None


def _get_nc():
    global _NC
    if _NC is None:
        _NC = build_nc()
    return _NC


def kernel(**inputs):
    q = np.asarray(inputs["q"], np.float32)
    k = np.asarray(inputs["k"], np.float32)
    v = np.asarray(inputs["v"], np.float32)
    Wq = np.asarray(inputs["Wq"], np.float32)
    Wk = np.asarray(inputs["Wk"], np.float32)
    Wv = np.asarray(inputs["Wv"], np.float32)
    Wo = np.asarray(inputs["Wo"], np.float32)
    bq = np.asarray(inputs["bq"], np.float32)
    bk = np.asarray(inputs["bk"], np.float32)
    bv = np.asarray(inputs["bv"], np.float32)
    bo = np.asarray(inputs["bo"], np.float32)

    nc = _get_nc()

    WqT = np.ascontiguousarray(Wq.T)
    WkT = np.ascontiguousarray(Wk.T)
    WvT = np.ascontiguousarray(Wv.T)
    WoT = np.ascontiguousarray(Wo.T)
    bqt = np.ascontiguousarray(bq.reshape(4, 128).T)
    bkt = np.ascontiguousarray(bk.reshape(4, 128).T)
    bot = np.ascontiguousarray(bo.reshape(4, 128).T)
    bvr = np.empty((1, D + 128), np.float32)
    bvr[0, :D] = bv
    bvr[0, D:] = 1.0
    idn = np.eye(128, dtype=np.float32)

    qTs = [np.ascontiguousarray(q[b].T) for b in range(B)]
    kTs = [np.ascontiguousarray(k[b].T) for b in range(B)]
    vTs = [np.ascontiguousarray(v[b].T) for b in range(B)]

    in_maps = []
    for c in range(N_CORES):
        b, lh = c // 2, c % 2
        in_maps.append(
            {
                "qT": np.ascontiguousarray(qTs[b][:, lh * Q : (lh + 1) * Q]),
                "kT": kTs[b],
                "vT": vTs[b],
                "WqT": WqT,
                "WkT": WkT,
                "WvT": WvT,
                "WoT": WoT,
                "bqt": bqt,
                "bkt": bkt,
                "bot": bot,
                "bvr": bvr,
                "idn": idn,
            }
        )

    res = run_bass_kernel_spmd(nc, in_maps, core_ids=list(range(N_CORES)))

    out = np.empty((B, L, D), np.float32)
    for c in range(N_CORES):
        b, lh = c // 2, c % 2
        out[b, lh * Q : (lh + 1) * Q, :] = res.results[c]["outT"].T
    return out


# revision 22
# speedup vs baseline: 1.2795x; 1.2795x over previous
"""MultiHeadAttention Trainium2 Bass kernel.

Reference math (B=4, L=2048, D=512, H=8, HD=64):
    qh = relu(q @ Wq.T + bq) ; kh = relu(k @ Wk.T + bk) ; vh = relu(v @ Wv.T + bv)
    scores = (qh_heads @ kh_heads.T) / sqrt(512)
    out = softmax(scores) @ vh_heads   (per head)
    out = relu(concat_heads(out) @ Wo.T + bo)
rel_k_table / rel_v_table are dead inputs.

Sharding: 8 cores = (batch b in 0..3) x (query-half lh in 0..1).
Each core computes the full output rows [b, lh*1024:(lh+1)*1024, :].
k/v projections are duplicated between the 2 cores of a batch (cheap) so
there is NO cross-core communication; host only transposes/concats.

On-device layout is fully transposed ("feature dim on partitions"):
  inputs uploaded as qT [512,1024], kT [512,2048], vT [512,2048] (host transpose)
  qhT/khT: [64*H rows, L] computed as  W.T-tiles (lhsT) @ xT (rhs)   [f32r]
  vh:      natural [l, do] via xT-tiles (lhsT) @ WvT (rhs), bias via ones-row
           matmul, relu on DVE -> bf16, augmented with a ones column (65 wide)
  scores_T[k, q] per head = khT-tile (lhsT, K=dh=64) @ qhT (rhs)     [f32r]
  P = exp(scores * 1/sqrt(512)) on ACT (N=1024 per instr) -> bf16
  attnV natural: P_T-tile (lhsT) @ vh_aug (rhs, N=65) -> [q,64|denom] [bf16]
  normalize rows by 1/denom (per-partition scalar on DVE), PE-transpose
  final_T = WoT-tiles (lhsT) @ oVT (rhs) + relu/bias                 [f32r]
  output stored transposed [512, 1024]; host transposes back.
"""

import sys

sys.path.insert(0, "/opt/trn_rl_repo")

import numpy as np

import concourse.bass as bass
import concourse.mybir as mybir
from concourse.tile import TileContext
from concourse.bass_utils import run_bass_kernel_spmd

B, L, D, H = 4, 2048, 512, 8
HD = D // H  # 64
Q = L // 2  # queries per core (1024)
SCALE = 1.0 / float(np.sqrt(D))

F32 = mybir.dt.float32
F32R = mybir.dt.float32r
BF16 = mybir.dt.bfloat16

N_CORES = 8


def _split_sem_waits(nc, max_waits=1):
    """walrus in this container only accepts one sem-wait per instruction;
    split extra waits onto preceding NoOps on the same engine."""
    ctr = [0]

    def mknop(engine, waits):
        ctr[0] += 1
        n = mybir.InstNoOp(name=f"I-waitfix-{ctr[0]}", ins=[], outs=[])
        n.engine = engine
        n.sync_info = mybir.SyncInfo(on_wait=list(waits), on_update=[])
        return n

    for fn in nc.m.functions:
        for bb in fn.blocks:
            changed = False
            new = []
            for inst in bb.instructions:
                si = inst.sync_info
                if si is not None and si.on_wait and len(si.on_wait) > max_waits:
                    waits = list(si.on_wait)
                    extra, keep = waits[:-max_waits], waits[-max_waits:]
                    for i in range(0, len(extra), max_waits):
                        new.append(mknop(inst.engine, extra[i : i + max_waits]))
                    inst.sync_info = mybir.SyncInfo(
                        on_wait=keep, on_update=list(si.on_update)
                    )
                    changed = True
                new.append(inst)
            if changed:
                bb.instructions = new


def _raise_sbuf_limit():
    try:
        from concourse import tile_utils

        if getattr(tile_utils, "max_sbuf_usage", 0) < 206 * 1024:
            tile_utils.max_sbuf_usage = 206 * 1024
    except Exception:
        pass


def build_nc(waitfix=True):
    _raise_sbuf_limit()
    nc = bass.Bass()

    qT = nc.declare_dram_parameter("qT", [D, Q], BF16, isOutput=False)
    kT = nc.declare_dram_parameter("kT", [D, L], BF16, isOutput=False)
    vT = nc.declare_dram_parameter("vT", [D, L], BF16, isOutput=False)
    WqT = nc.declare_dram_parameter("WqT", [D, D], BF16, isOutput=False)
    WkT = nc.declare_dram_parameter("WkT", [D, D], BF16, isOutput=False)
    WvT = nc.declare_dram_parameter("WvT", [D, D], BF16, isOutput=False)
    WoT = nc.declare_dram_parameter("WoT", [D, D], F32R, isOutput=False)
    # biases pre-tiled on host: [128, 4] column t = bias slice for do-tile t
    bqt = nc.declare_dram_parameter("bqt", [128, 4], F32, isOutput=False)
    bkt = nc.declare_dram_parameter("bkt", [128, 4], F32, isOutput=False)
    bot = nc.declare_dram_parameter("bot", [128, 4], F32, isOutput=False)
    bvr = nc.declare_dram_parameter("bvr", [1, D + 128], F32R, isOutput=False)
    idn = nc.declare_dram_parameter("idn", [128, 128], F32, isOutput=False)
    outT = nc.declare_dram_parameter("outT", [D, Q], F32, isOutput=True)

    # dram views with row-tiles on partitions
    qT_t = qT.rearrange("(t p) l -> p t l", p=128)  # [128, 4, Q]
    kT_t = kT.rearrange("(t p) l -> p t l", p=128)  # [128, 4, L]
    vT_t = vT.rearrange("(t p) l -> p t l", p=128)
    WqT_t = WqT.rearrange("(t p) d -> p t d", p=128)  # [128, 4, 512]
    WkT_t = WkT.rearrange("(t p) d -> p t d", p=128)
    WvT_t = WvT.rearrange("(t p) d -> p t d", p=128)
    WoT_t = WoT.rearrange("(t p) d -> p t d", p=128)
    outT_t = outT.rearrange("(t p) l -> p t l", p=128)

    KT = L // 128  # 16 k-tiles
    QT = Q // 128  # 8 q-tiles

    with TileContext(nc) as tc:
        with (
            tc.tile_pool(name="persist", bufs=1) as persist,
            tc.tile_pool(name="weights", bufs=1) as wpool,
            tc.tile_pool(name="stream", bufs=2) as stream,
            tc.tile_pool(name="pslab", bufs=2) as pslab,
            tc.tile_pool(name="outp", bufs=2) as outp,
            tc.tile_pool(name="ps_big", bufs=3, space="PSUM") as ps_big,
            tc.tile_pool(name="ps_small", bufs=2, space="PSUM") as ps_small,
        ):
            # ---- constants / weights ----
            w_q = wpool.tile([128, 4, D], BF16, tag="wq")
            w_k = wpool.tile([128, 4, D], BF16, tag="wk")
            w_v = wpool.tile([128, 4, D], BF16, tag="wv")
            w_o = persist.tile([128, 4, D], F32R, tag="wo")
            nc.sync.dma_start(out=w_k, in_=WkT_t)
            nc.sync.dma_start(out=w_q, in_=WqT_t)
            nc.sync.dma_start(out=w_v, in_=WvT_t)
            nc.sync.dma_start(out=w_o, in_=WoT_t)
            b_q = persist.tile([128, 4], F32, tag="bq")
            b_k = persist.tile([128, 4], F32, tag="bk")
            b_o = persist.tile([128, 4], F32, tag="bo")
            b_v = persist.tile([1, D + 128], F32R, tag="bv")
            nc.sync.dma_start(out=b_q, in_=bqt[:, :])
            nc.sync.dma_start(out=b_k, in_=bkt[:, :])
            nc.sync.dma_start(out=b_o, in_=bot[:, :])
            nc.sync.dma_start(out=b_v, in_=bvr[:, :])
            ones_row = b_v[0:1, D : D + 128]

            # ---- persistent activations ----
            # khT / qhT: 4 tiles of [128, L] fp32; partition = feature (2 heads/tile)
            khT = [persist.tile([128, L], BF16, tag=f"khT{t}", name=f"khT{t}") for t in range(4)]
            qhT = [persist.tile([128, Q], BF16, tag=f"qhT{t}", name=f"qhT{t}") for t in range(4)]
            # vh natural, bf16, augmented ones col: [128, kt, h, 65]
            vh = persist.tile([128, KT, H, HD + 1], BF16, tag="vh")
            nc.vector.memset(vh[:, :, :, HD : HD + 1], 1.0)
            # attention output transposed: 4 tiles [128, Q] fp32
            oVT = [persist.tile([128, Q], F32R, tag=f"oVT{t}", name=f"oVT{t}") for t in range(4)]

            # ---- projections ----
            # khT[dt][:, :] = relu(WkT_tiles.T @ kT + bk); stream kT in 4 chunks
            LC = 4  # l-chunks of 512 for k/v
            for lc in range(LC):
                kc = stream.tile([128, 4, 512], BF16, tag="chunk")
                nc.sync.dma_start(out=kc, in_=kT_t[:, :, lc * 512 : (lc + 1) * 512])
                for dt in range(4):
                    ps = ps_small.tile([128, 512], F32, tag="mm")
                    for kt in range(4):
                        nc.tensor.matmul(
                            ps,
                            lhsT=w_k[:, kt, dt * 128 : (dt + 1) * 128],
                            rhs=kc[:, kt, :],
                            start=(kt == 0),
                            stop=(kt == 3),
                        )
                    nc.vector.tensor_scalar(
                        out=khT[dt][:, lc * 512 : (lc + 1) * 512],
                        in0=ps,
                        scalar1=b_k[:, dt : dt + 1],
                        scalar2=0.0,
                        op0=mybir.AluOpType.add,
                        op1=mybir.AluOpType.max,
                    )
                # vh natural for this chunk: 4 l-tiles of 128
                vc = stream.tile([128, 4, 512], BF16, tag="chunk")
                nc.sync.dma_start(out=vc, in_=vT_t[:, :, lc * 512 : (lc + 1) * 512])
                for lt4 in range(4):
                    lt = lc * 4 + lt4  # global l-tile (= k-tile index)
                    ps = ps_small.tile([128, 512], F32, tag="mm")
                    for kt in range(4):
                        nc.tensor.matmul(
                            ps,
                            lhsT=vc[:, kt, lt4 * 128 : (lt4 + 1) * 128],
                            rhs=w_v[:, kt, :],
                            start=(kt == 0),
                            stop=False,
                        )
                    nc.tensor.matmul(
                        ps,
                        lhsT=ones_row,
                        rhs=b_v[0:1, 0:D],
                        start=False,
                        stop=True,
                    )
                    nc.vector.tensor_scalar(
                        out=vh[:, lt, :, 0:HD],
                        in0=ps.rearrange("p (h d) -> p h d", h=H),
                        scalar1=0.0,
                        scalar2=None,
                        op0=mybir.AluOpType.max,
                    )
            for lc in range(2):  # q chunks (Q=1024)
                qc = stream.tile([128, 4, 512], BF16, tag="chunk")
                nc.sync.dma_start(out=qc, in_=qT_t[:, :, lc * 512 : (lc + 1) * 512])
                for dt in range(4):
                    ps = ps_small.tile([128, 512], F32, tag="mm")
                    for kt in range(4):
                        nc.tensor.matmul(
                            ps,
                            lhsT=w_q[:, kt, dt * 128 : (dt + 1) * 128],
                            rhs=qc[:, kt, :],
                            start=(kt == 0),
                            stop=(kt == 3),
                        )
                    nc.vector.tensor_scalar(
                        out=qhT[dt][:, lc * 512 : (lc + 1) * 512],
                        in0=ps,
                        scalar1=b_q[:, dt : dt + 1],
                        scalar2=0.0,
                        op0=mybir.AluOpType.add,
                        op1=mybir.AluOpType.max,
                    )

            # ---- attention per head ----
            # Normalization runs one (h,qch) iteration behind the attnV
            # matmul groups so the PE stream never waits on DVE.
            def emit_norm(pend):
                rc, oU, tht, tho, qsl = pend
                ps_b = ps_small.tile([HD, 512], F32, tag="mm")
                nc.tensor.matmul(
                    ps_b,
                    lhsT=ones_row[0:1, 0:HD],
                    rhs=rc,
                    start=True,
                    stop=True,
                )
                recipB = outp.tile([HD, 512], F32, tag="recipB", bufs=1)
                nc.vector.tensor_copy(out=recipB, in_=ps_b)
                nc.vector.tensor_tensor(
                    out=oVT[tht][tho : tho + 64, qsl],
                    in0=oU,
                    in1=recipB,
                    op=mybir.AluOpType.mult,
                )

            pend = None
            for h in range(H):
                ht, ho = h // 2, (h % 2) * 64
                P = pslab.tile([128, KT, Q], BF16, tag="P")
                for kt in range(KT):
                    ps_s = ps_big.tile([128, Q], F32, tag="scores")
                    for qh2 in range(2):
                        nc.tensor.matmul(
                            ps_s[:, qh2 * 512 : (qh2 + 1) * 512],
                            lhsT=khT[ht][ho : ho + 64, kt * 128 : (kt + 1) * 128],
                            rhs=qhT[ht][ho : ho + 64, qh2 * 512 : (qh2 + 1) * 512],
                            start=True,
                            stop=True,
                        )
                    nc.scalar.activation(
                        out=P[:, kt, :],
                        in_=ps_s,
                        func=mybir.ActivationFunctionType.Exp,
                        scale=SCALE,
                    )
                for qch in range(2):
                    qsl = slice(qch * 512, (qch + 1) * 512)
                    ps_av = ps_small.tile([HD + 1, 512], F32, tag="mm")
                    for kt in range(KT):
                        nc.tensor.matmul(
                            ps_av,
                            lhsT=vh[:, kt, h, :],
                            rhs=P[:, kt, qsl],
                            start=(kt == 0),
                            stop=(kt == KT - 1),
                        )
                    oU = outp.tile([HD, 512], F32, tag="oU")
                    nc.vector.tensor_copy(out=oU, in_=ps_av[0:HD, :])
                    rc = outp.tile([1, 512], F32R, tag="fout")
                    with nc.allow_low_precision(reason="f32r recip bcast"):
                        nc.vector.reciprocal(out=rc, in_=ps_av[HD : HD + 1, :])
                    if pend is not None:
                        emit_norm(pend)
                    pend = (rc, oU, ht, ho, qsl)
            emit_norm(pend)

            # ---- output projection ----
            for qc2 in range(2):
                for dt in range(4):
                    ps = ps_small.tile([128, 512], F32, tag="mm")
                    for ktile in range(4):
                        nc.tensor.matmul(
                            ps,
                            lhsT=w_o[:, ktile, dt * 128 : (dt + 1) * 128],
                            rhs=oVT[ktile][:, qc2 * 512 : (qc2 + 1) * 512],
                            start=(ktile == 0),
                            stop=(ktile == 3),
                        )
                    fo = outp.tile([128, 512], F32, tag="fout")
                    nc.vector.tensor_scalar(
                        out=fo,
                        in0=ps,
                        scalar1=b_o[:, dt : dt + 1],
                        scalar2=0.0,
                        op0=mybir.AluOpType.add,
                        op1=mybir.AluOpType.max,
                    )
                    nc.sync.dma_start(
                        out=outT_t[:, dt, qc2 * 512 : (qc2 + 1) * 512], in_=fo
                    )

    if waitfix:
        _split_sem_waits(nc, max_waits=1)
    return nc


_NC = None


def _get_nc():
    global _NC
    if _NC is None:
        _NC = build_nc()
    return _NC


def kernel(**inputs):
    q = np.asarray(inputs["q"], np.float32)
    k = np.asarray(inputs["k"], np.float32)
    v = np.asarray(inputs["v"], np.float32)
    Wq = np.asarray(inputs["Wq"], np.float32)
    Wk = np.asarray(inputs["Wk"], np.float32)
    Wv = np.asarray(inputs["Wv"], np.float32)
    Wo = np.asarray(inputs["Wo"], np.float32)
    bq = np.asarray(inputs["bq"], np.float32)
    bk = np.asarray(inputs["bk"], np.float32)
    bv = np.asarray(inputs["bv"], np.float32)
    bo = np.asarray(inputs["bo"], np.float32)

    nc = _get_nc()

    import ml_dtypes

    bf16 = ml_dtypes.bfloat16
    WqT = np.ascontiguousarray(Wq.T).astype(bf16)
    WkT = np.ascontiguousarray(Wk.T).astype(bf16)
    WvT = np.ascontiguousarray(Wv.T).astype(bf16)
    WoT = np.ascontiguousarray(Wo.T)
    bqt = np.ascontiguousarray(bq.reshape(4, 128).T)
    bkt = np.ascontiguousarray(bk.reshape(4, 128).T)
    bot = np.ascontiguousarray(bo.reshape(4, 128).T)
    bvr = np.empty((1, D + 128), np.float32)
    bvr[0, :D] = bv
    bvr[0, D:] = 1.0
    idn = np.eye(128, dtype=np.float32)

    qTs = [np.ascontiguousarray(q[b].T).astype(bf16) for b in range(B)]
    kTs = [np.ascontiguousarray(k[b].T).astype(bf16) for b in range(B)]
    vTs = [np.ascontiguousarray(v[b].T).astype(bf16) for b in range(B)]

    in_maps = []
    for c in range(N_CORES):
        b, lh = c // 2, c % 2
        in_maps.append(
            {
                "qT": np.ascontiguousarray(qTs[b][:, lh * Q : (lh + 1) * Q]),
                "kT": kTs[b],
                "vT": vTs[b],
                "WqT": WqT,
                "WkT": WkT,
                "WvT": WvT,
                "WoT": WoT,
                "bqt": bqt,
                "bkt": bkt,
                "bot": bot,
                "bvr": bvr,
                "idn": idn,
            }
        )

    res = run_bass_kernel_spmd(nc, in_maps, core_ids=list(range(N_CORES)))

    out = np.empty((B, L, D), np.float32)
    for c in range(N_CORES):
        b, lh = c // 2, c % 2
        out[b, lh * Q : (lh + 1) * Q, :] = res.results[c]["outT"].T
    return out


# revision 25
# speedup vs baseline: 1.3941x; 1.0896x over previous
"""MultiHeadAttention Trainium2 Bass kernel.

Reference math (B=4, L=2048, D=512, H=8, HD=64):
    qh = relu(q @ Wq.T + bq) ; kh = relu(k @ Wk.T + bk) ; vh = relu(v @ Wv.T + bv)
    scores = (qh_heads @ kh_heads.T) / sqrt(512)
    out = softmax(scores) @ vh_heads   (per head)
    out = relu(concat_heads(out) @ Wo.T + bo)
rel_k_table / rel_v_table are dead inputs.

Sharding: 8 cores = (batch b in 0..3) x (query-half lh in 0..1).
Each core computes the full output rows [b, lh*1024:(lh+1)*1024, :].
k/v projections are duplicated between the 2 cores of a batch (cheap) so
there is NO cross-core communication; host only transposes/concats.

On-device layout is fully transposed ("feature dim on partitions"):
  inputs uploaded as qT [512,1024], kT [512,2048], vT [512,2048] (host transpose)
  qhT/khT: [64*H rows, L] computed as  W.T-tiles (lhsT) @ xT (rhs)   [f32r]
  vh:      natural [l, do] via xT-tiles (lhsT) @ WvT (rhs), bias via ones-row
           matmul, relu on DVE -> bf16, augmented with a ones column (65 wide)
  scores_T[k, q] per head = khT-tile (lhsT, K=dh=64) @ qhT (rhs)     [f32r]
  P = exp(scores * 1/sqrt(512)) on ACT (N=1024 per instr) -> bf16
  attnV natural: P_T-tile (lhsT) @ vh_aug (rhs, N=65) -> [q,64|denom] [bf16]
  normalize rows by 1/denom (per-partition scalar on DVE), PE-transpose
  final_T = WoT-tiles (lhsT) @ oVT (rhs) + relu/bias                 [f32r]
  output stored transposed [512, 1024]; host transposes back.
"""

import sys

sys.path.insert(0, "/opt/trn_rl_repo")

import numpy as np

import concourse.bass as bass
import concourse.mybir as mybir
from concourse.tile import TileContext
from concourse.bass_utils import run_bass_kernel_spmd

B, L, D, H = 4, 2048, 512, 8
HD = D // H  # 64
Q = L // 2  # queries per core (1024)
SCALE = 1.0 / float(np.sqrt(D))

F32 = mybir.dt.float32
F32R = mybir.dt.float32r
BF16 = mybir.dt.bfloat16

N_CORES = 8


def _split_sem_waits(nc, max_waits=1):
    """walrus in this container only accepts one sem-wait per instruction;
    split extra waits onto preceding NoOps on the same engine."""
    ctr = [0]

    def mknop(engine, waits):
        ctr[0] += 1
        n = mybir.InstNoOp(name=f"I-waitfix-{ctr[0]}", ins=[], outs=[])
        n.engine = engine
        n.sync_info = mybir.SyncInfo(on_wait=list(waits), on_update=[])
        return n

    for fn in nc.m.functions:
        for bb in fn.blocks:
            changed = False
            new = []
            for inst in bb.instructions:
                si = inst.sync_info
                if si is not None and si.on_wait and len(si.on_wait) > max_waits:
                    waits = list(si.on_wait)
                    extra, keep = waits[:-max_waits], waits[-max_waits:]
                    for i in range(0, len(extra), max_waits):
                        new.append(mknop(inst.engine, extra[i : i + max_waits]))
                    inst.sync_info = mybir.SyncInfo(
                        on_wait=keep, on_update=list(si.on_update)
                    )
                    changed = True
                new.append(inst)
            if changed:
                bb.instructions = new


def _raise_sbuf_limit():
    try:
        from concourse import tile_utils

        if getattr(tile_utils, "max_sbuf_usage", 0) < 206 * 1024:
            tile_utils.max_sbuf_usage = 206 * 1024
    except Exception:
        pass


def build_nc(waitfix=True):
    _raise_sbuf_limit()
    nc = bass.Bass()

    qT = nc.declare_dram_parameter("qT", [D, Q], BF16, isOutput=False)
    kT = nc.declare_dram_parameter("kT", [D, L], BF16, isOutput=False)
    vT = nc.declare_dram_parameter("vT", [D, L], BF16, isOutput=False)
    WqT = nc.declare_dram_parameter("WqT", [D, D], BF16, isOutput=False)
    WkT = nc.declare_dram_parameter("WkT", [D, D], BF16, isOutput=False)
    WvT = nc.declare_dram_parameter("WvT", [D, D], BF16, isOutput=False)
    WoT = nc.declare_dram_parameter("WoT", [D, D], F32R, isOutput=False)
    # biases pre-tiled on host: [128, 4] column t = bias slice for do-tile t
    bqt = nc.declare_dram_parameter("bqt", [128, 4], F32, isOutput=False)
    bkt = nc.declare_dram_parameter("bkt", [128, 4], F32, isOutput=False)
    bot = nc.declare_dram_parameter("bot", [128, 4], F32, isOutput=False)
    bvr = nc.declare_dram_parameter("bvr", [1, D + 128], F32R, isOutput=False)
    idn = nc.declare_dram_parameter("idn", [128, 128], F32, isOutput=False)
    outT = nc.declare_dram_parameter("outT", [D, Q], F32, isOutput=True)

    # dram views with row-tiles on partitions
    qT_t = qT.rearrange("(t p) l -> p t l", p=128)  # [128, 4, Q]
    kT_t = kT.rearrange("(t p) l -> p t l", p=128)  # [128, 4, L]
    vT_t = vT.rearrange("(t p) l -> p t l", p=128)
    WqT_t = WqT.rearrange("(t p) d -> p t d", p=128)  # [128, 4, 512]
    WkT_t = WkT.rearrange("(t p) d -> p t d", p=128)
    WvT_t = WvT.rearrange("(t p) d -> p t d", p=128)
    WoT_t = WoT.rearrange("(t p) d -> p t d", p=128)
    outT_t = outT.rearrange("(t p) l -> p t l", p=128)

    KT = L // 128  # 16 k-tiles
    QT = Q // 128  # 8 q-tiles

    with TileContext(nc) as tc:
        with (
            tc.tile_pool(name="persist", bufs=1) as persist,
            tc.tile_pool(name="weights", bufs=1) as wpool,
            tc.tile_pool(name="stream", bufs=2) as stream,
            tc.tile_pool(name="pslab", bufs=2) as pslab,
            tc.tile_pool(name="outp", bufs=2) as outp,
            tc.tile_pool(name="ps_big", bufs=3, space="PSUM") as ps_big,
            tc.tile_pool(name="ps_small", bufs=2, space="PSUM") as ps_small,
        ):
            # ---- constants / weights ----
            # Load order matters: w_k + first kT chunk gate the first matmul,
            # so they go first; everything else is needed later.
            w_q = wpool.tile([128, 4, D], BF16, tag="wq")
            w_k = wpool.tile([128, 4, D], BF16, tag="wk")
            w_v = wpool.tile([128, 4, D], BF16, tag="wv")
            w_o = persist.tile([128, 4, D], F32R, tag="wo")
            b_q = persist.tile([128, 4], F32, tag="bq")
            b_k = persist.tile([128, 4], F32, tag="bk")
            b_o = persist.tile([128, 4], F32, tag="bo")
            b_v = persist.tile([1, D + 128], F32R, tag="bv")
            nc.sync.dma_start(out=w_k, in_=WkT_t)
            nc.sync.dma_start(out=b_k, in_=bkt[:, :])
            ones_row = b_v[0:1, D : D + 128]

            # ---- persistent activations ----
            # khT / qhT: 4 tiles of [128, L] fp32; partition = feature (2 heads/tile)
            khT = [persist.tile([128, L], BF16, tag=f"khT{t}", name=f"khT{t}") for t in range(4)]
            qhT = [persist.tile([128, Q], BF16, tag=f"qhT{t}", name=f"qhT{t}") for t in range(4)]
            # vh natural, bf16, augmented ones col: [128, kt, h, 65]
            vh = persist.tile([128, KT, H, HD + 1], BF16, tag="vh")
            nc.vector.memset(vh[:, :, :, HD : HD + 1], 1.0)
            # attention output transposed: 4 tiles [128, Q] fp32
            oVT = [persist.tile([128, Q], F32R, tag=f"oVT{t}", name=f"oVT{t}") for t in range(4)]

            # ---- projections ----
            # khT[dt][:, :] = relu(WkT_tiles.T @ kT + bk); stream kT in 4 chunks
            LC = 4  # l-chunks of 512 for k/v
            for lc in range(LC):
                kc = stream.tile([128, 4, 512], BF16, tag="chunk")
                nc.sync.dma_start(out=kc, in_=kT_t[:, :, lc * 512 : (lc + 1) * 512])
                if lc == 0:
                    nc.sync.dma_start(out=w_v, in_=WvT_t)
                    nc.sync.dma_start(out=b_v, in_=bvr[:, :])
                elif lc == 1:
                    nc.sync.dma_start(out=w_q, in_=WqT_t)
                    nc.sync.dma_start(out=b_q, in_=bqt[:, :])
                elif lc == 2:
                    nc.sync.dma_start(out=w_o, in_=WoT_t)
                    nc.sync.dma_start(out=b_o, in_=bot[:, :])
                for dt in range(4):
                    ps = ps_small.tile([128, 512], F32, tag="mm")
                    for kt in range(4):
                        nc.tensor.matmul(
                            ps,
                            lhsT=w_k[:, kt, dt * 128 : (dt + 1) * 128],
                            rhs=kc[:, kt, :],
                            start=(kt == 0),
                            stop=(kt == 3),
                        )
                    nc.vector.tensor_scalar(
                        out=khT[dt][:, lc * 512 : (lc + 1) * 512],
                        in0=ps,
                        scalar1=b_k[:, dt : dt + 1],
                        scalar2=0.0,
                        op0=mybir.AluOpType.add,
                        op1=mybir.AluOpType.max,
                    )
                # vh natural for this chunk: 4 l-tiles of 128
                vc = stream.tile([128, 4, 512], BF16, tag="chunk")
                nc.sync.dma_start(out=vc, in_=vT_t[:, :, lc * 512 : (lc + 1) * 512])
                for lt4 in range(4):
                    lt = lc * 4 + lt4  # global l-tile (= k-tile index)
                    ps = ps_small.tile([128, 512], F32, tag="mm")
                    for kt in range(4):
                        nc.tensor.matmul(
                            ps,
                            lhsT=vc[:, kt, lt4 * 128 : (lt4 + 1) * 128],
                            rhs=w_v[:, kt, :],
                            start=(kt == 0),
                            stop=False,
                        )
                    nc.tensor.matmul(
                        ps,
                        lhsT=ones_row,
                        rhs=b_v[0:1, 0:D],
                        start=False,
                        stop=True,
                    )
                    nc.vector.tensor_scalar(
                        out=vh[:, lt, :, 0:HD],
                        in0=ps.rearrange("p (h d) -> p h d", h=H),
                        scalar1=0.0,
                        scalar2=None,
                        op0=mybir.AluOpType.max,
                    )
            for lc in range(2):  # q chunks (Q=1024)
                qc = stream.tile([128, 4, 512], BF16, tag="chunk")
                nc.sync.dma_start(out=qc, in_=qT_t[:, :, lc * 512 : (lc + 1) * 512])
                for dt in range(4):
                    ps = ps_small.tile([128, 512], F32, tag="mm")
                    for kt in range(4):
                        nc.tensor.matmul(
                            ps,
                            lhsT=w_q[:, kt, dt * 128 : (dt + 1) * 128],
                            rhs=qc[:, kt, :],
                            start=(kt == 0),
                            stop=(kt == 3),
                        )
                    nc.vector.tensor_scalar(
                        out=qhT[dt][:, lc * 512 : (lc + 1) * 512],
                        in0=ps,
                        scalar1=b_q[:, dt : dt + 1],
                        scalar2=0.0,
                        op0=mybir.AluOpType.add,
                        op1=mybir.AluOpType.max,
                    )

            # ---- attention per head ----
            # Normalization runs one (h,qch) iteration behind the attnV
            # matmul groups so the PE stream never waits on DVE.
            def emit_norm(pend):
                rc, oU, tht, tho, qsl = pend
                ps_b = ps_small.tile([HD, 512], F32, tag="mm")
                nc.tensor.matmul(
                    ps_b,
                    lhsT=ones_row[0:1, 0:HD],
                    rhs=rc,
                    start=True,
                    stop=True,
                )
                recipB = outp.tile([HD, 512], F32, tag="recipB", bufs=1)
                nc.vector.tensor_copy(out=recipB, in_=ps_b)
                nc.vector.tensor_tensor(
                    out=oVT[tht][tho : tho + 64, qsl],
                    in0=oU,
                    in1=recipB,
                    op=mybir.AluOpType.mult,
                )

            P_tiles = {}
            state = {"pend": None}

            def emit_scores(h, kts):
                ht, ho = h // 2, (h % 2) * 64
                P = P_tiles[h]
                for kt in kts:
                    ps_s = ps_big.tile([128, Q], F32, tag="scores")
                    for qh2 in range(2):
                        nc.tensor.matmul(
                            ps_s[:, qh2 * 512 : (qh2 + 1) * 512],
                            lhsT=khT[ht][ho : ho + 64, kt * 128 : (kt + 1) * 128],
                            rhs=qhT[ht][ho : ho + 64, qh2 * 512 : (qh2 + 1) * 512],
                            start=True,
                            stop=True,
                        )
                    nc.scalar.activation(
                        out=P[:, kt, :],
                        in_=ps_s,
                        func=mybir.ActivationFunctionType.Exp,
                        scale=SCALE,
                    )

            def emit_attnv(h, qch):
                ht, ho = h // 2, (h % 2) * 64
                qsl = slice(qch * 512, (qch + 1) * 512)
                P = P_tiles[h]
                ps_av = ps_small.tile([HD + 1, 512], F32, tag="mm")
                for kt in range(KT):
                    nc.tensor.matmul(
                        ps_av,
                        lhsT=vh[:, kt, h, :],
                        rhs=P[:, kt, qsl],
                        start=(kt == 0),
                        stop=(kt == KT - 1),
                    )
                oU = outp.tile([HD, 512], F32, tag="oU")
                nc.vector.tensor_copy(out=oU, in_=ps_av[0:HD, :])
                rc = outp.tile([1, 512], F32R, tag="fout")
                with nc.allow_low_precision(reason="f32r recip bcast"):
                    nc.vector.reciprocal(out=rc, in_=ps_av[HD : HD + 1, :])
                if state["pend"] is not None:
                    emit_norm(state["pend"])
                state["pend"] = (rc, oU, ht, ho, qsl)

            # software pipeline: scores(h+1) halves interleave with attnV(h)
            # chunks so ACT (exp) always has fresh score tiles to chew on.
            P_tiles[0] = pslab.tile([128, KT, Q], BF16, tag="P", name="P0")
            emit_scores(0, range(KT))
            for h in range(H):
                if h + 1 < H:
                    P_tiles[h + 1] = pslab.tile(
                        [128, KT, Q], BF16, tag="P", name=f"P{h + 1}"
                    )
                    emit_scores(h + 1, range(0, KT // 2))
                emit_attnv(h, 0)
                if h + 1 < H:
                    emit_scores(h + 1, range(KT // 2, KT))
                emit_attnv(h, 1)
                del P_tiles[h]
            emit_norm(state["pend"])

            # ---- output projection ----
            for qc2 in range(2):
                for dt in range(4):
                    ps = ps_small.tile([128, 512], F32, tag="mm")
                    for ktile in range(4):
                        nc.tensor.matmul(
                            ps,
                            lhsT=w_o[:, ktile, dt * 128 : (dt + 1) * 128],
                            rhs=oVT[ktile][:, qc2 * 512 : (qc2 + 1) * 512],
                            start=(ktile == 0),
                            stop=(ktile == 3),
                        )
                    fo = outp.tile([128, 512], F32, tag="fout")
                    nc.vector.tensor_scalar(
                        out=fo,
                        in0=ps,
                        scalar1=b_o[:, dt : dt + 1],
                        scalar2=0.0,
                        op0=mybir.AluOpType.add,
                        op1=mybir.AluOpType.max,
                    )
                    nc.sync.dma_start(
                        out=outT_t[:, dt, qc2 * 512 : (qc2 + 1) * 512], in_=fo
                    )

    if waitfix:
        _split_sem_waits(nc, max_waits=1)
    return nc


_NC = None


def _get_nc():
    global _NC
    if _NC is None:
        _NC = build_nc()
    return _NC


def kernel(**inputs):
    q = np.asarray(inputs["q"], np.float32)
    k = np.asarray(inputs["k"], np.float32)
    v = np.asarray(inputs["v"], np.float32)
    Wq = np.asarray(inputs["Wq"], np.float32)
    Wk = np.asarray(inputs["Wk"], np.float32)
    Wv = np.asarray(inputs["Wv"], np.float32)
    Wo = np.asarray(inputs["Wo"], np.float32)
    bq = np.asarray(inputs["bq"], np.float32)
    bk = np.asarray(inputs["bk"], np.float32)
    bv = np.asarray(inputs["bv"], np.float32)
    bo = np.asarray(inputs["bo"], np.float32)

    nc = _get_nc()

    import ml_dtypes

    bf16 = ml_dtypes.bfloat16
    WqT = np.ascontiguousarray(Wq.T).astype(bf16)
    WkT = np.ascontiguousarray(Wk.T).astype(bf16)
    WvT = np.ascontiguousarray(Wv.T).astype(bf16)
    WoT = np.ascontiguousarray(Wo.T)
    bqt = np.ascontiguousarray(bq.reshape(4, 128).T)
    bkt = np.ascontiguousarray(bk.reshape(4, 128).T)
    bot = np.ascontiguousarray(bo.reshape(4, 128).T)
    bvr = np.empty((1, D + 128), np.float32)
    bvr[0, :D] = bv
    bvr[0, D:] = 1.0
    idn = np.eye(128, dtype=np.float32)

    qTs = [np.ascontiguousarray(q[b].T).astype(bf16) for b in range(B)]
    kTs = [np.ascontiguousarray(k[b].T).astype(bf16) for b in range(B)]
    vTs = [np.ascontiguousarray(v[b].T).astype(bf16) for b in range(B)]

    in_maps = []
    for c in range(N_CORES):
        b, lh = c // 2, c % 2
        in_maps.append(
            {
                "qT": np.ascontiguousarray(qTs[b][:, lh * Q : (lh + 1) * Q]),
                "kT": kTs[b],
                "vT": vTs[b],
                "WqT": WqT,
                "WkT": WkT,
                "WvT": WvT,
                "WoT": WoT,
                "bqt": bqt,
                "bkt": bkt,
                "bot": bot,
                "bvr": bvr,
                "idn": idn,
            }
        )

    res = run_bass_kernel_spmd(nc, in_maps, core_ids=list(range(N_CORES)))

    out = np.empty((B, L, D), np.float32)
    for c in range(N_CORES):
        b, lh = c // 2, c % 2
        out[b, lh * Q : (lh + 1) * Q, :] = res.results[c]["outT"].T
    return out


# revision 30
# speedup vs baseline: 1.5575x; 1.1172x over previous
"""MultiHeadAttention Trainium2 Bass kernel.

Reference math (B=4, L=2048, D=512, H=8, HD=64):
    qh = relu(q @ Wq.T + bq) ; kh = relu(k @ Wk.T + bk) ; vh = relu(v @ Wv.T + bv)
    scores = (qh_heads @ kh_heads.T) / sqrt(512)
    out = softmax(scores) @ vh_heads   (per head)
    out = relu(concat_heads(out) @ Wo.T + bo)
rel_k_table / rel_v_table are dead inputs.

Sharding: 8 cores = (batch b in 0..3) x (query-half lh in 0..1).
Each core computes the full output rows [b, lh*1024:(lh+1)*1024, :].
k/v projections are duplicated between the 2 cores of a batch (cheap) so
there is NO cross-core communication; host only transposes/concats.

On-device layout is fully transposed ("feature dim on partitions"):
  inputs uploaded as qT [512,1024], kT [512,2048], vT [512,2048] (host transpose)
  qhT/khT: [64*H rows, L] computed as  W.T-tiles (lhsT) @ xT (rhs)   [f32r]
  vh:      natural [l, do] via xT-tiles (lhsT) @ WvT (rhs), bias via ones-row
           matmul, relu on DVE -> bf16, augmented with a ones column (65 wide)
  scores_T[k, q] per head = khT-tile (lhsT, K=dh=64) @ qhT (rhs)     [f32r]
  P = exp(scores * 1/sqrt(512)) on ACT (N=1024 per instr) -> bf16
  attnV natural: P_T-tile (lhsT) @ vh_aug (rhs, N=65) -> [q,64|denom] [bf16]
  normalize rows by 1/denom (per-partition scalar on DVE), PE-transpose
  final_T = WoT-tiles (lhsT) @ oVT (rhs) + relu/bias                 [f32r]
  output stored transposed [512, 1024]; host transposes back.
"""

import sys

sys.path.insert(0, "/opt/trn_rl_repo")

import numpy as np

import concourse.bass as bass
import concourse.mybir as mybir
from concourse.tile import TileContext
from concourse.bass_utils import run_bass_kernel_spmd

B, L, D, H = 4, 2048, 512, 8
HD = D // H  # 64
Q = L // 2  # queries per core (1024)
SCALE = 1.0 / float(np.sqrt(D))

F32 = mybir.dt.float32
F32R = mybir.dt.float32r
BF16 = mybir.dt.bfloat16

N_CORES = 8


def _split_sem_waits(nc, max_waits=1):
    """walrus in this container only accepts one sem-wait per instruction;
    split extra waits onto preceding NoOps on the same engine."""
    ctr = [0]

    def mknop(engine, waits):
        ctr[0] += 1
        n = mybir.InstNoOp(name=f"I-waitfix-{ctr[0]}", ins=[], outs=[])
        n.engine = engine
        n.sync_info = mybir.SyncInfo(on_wait=list(waits), on_update=[])
        return n

    for fn in nc.m.functions:
        for bb in fn.blocks:
            changed = False
            new = []
            for inst in bb.instructions:
                si = inst.sync_info
                if si is not None and si.on_wait and len(si.on_wait) > max_waits:
                    waits = list(si.on_wait)
                    extra, keep = waits[:-max_waits], waits[-max_waits:]
                    for i in range(0, len(extra), max_waits):
                        new.append(mknop(inst.engine, extra[i : i + max_waits]))
                    inst.sync_info = mybir.SyncInfo(
                        on_wait=keep, on_update=list(si.on_update)
                    )
                    changed = True
                new.append(inst)
            if changed:
                bb.instructions = new


def _raise_sbuf_limit():
    try:
        from concourse import tile_utils

        if getattr(tile_utils, "max_sbuf_usage", 0) < 206 * 1024:
            tile_utils.max_sbuf_usage = 206 * 1024
    except Exception:
        pass


def build_nc(waitfix=True):
    _raise_sbuf_limit()
    nc = bass.Bass()

    qT = nc.declare_dram_parameter("qT", [D, Q], BF16, isOutput=False)
    kT = nc.declare_dram_parameter("kT", [D, L], BF16, isOutput=False)
    vT = nc.declare_dram_parameter("vT", [D, L], BF16, isOutput=False)
    WqT = nc.declare_dram_parameter("WqT", [D, D], BF16, isOutput=False)
    WkT = nc.declare_dram_parameter("WkT", [D, D], BF16, isOutput=False)
    WvT = nc.declare_dram_parameter("WvT", [D, D], BF16, isOutput=False)
    WoT = nc.declare_dram_parameter("WoT", [D, D], BF16, isOutput=False)
    # biases pre-tiled on host: [128, 4] column t = bias slice for do-tile t
    bqt = nc.declare_dram_parameter("bqt", [128, 4], F32, isOutput=False)
    bkt = nc.declare_dram_parameter("bkt", [128, 4], F32, isOutput=False)
    bot = nc.declare_dram_parameter("bot", [128, 4], F32, isOutput=False)
    bvr = nc.declare_dram_parameter("bvr", [1, D + 128], F32R, isOutput=False)
    idn = nc.declare_dram_parameter("idn", [128, 128], F32, isOutput=False)
    outT = nc.declare_dram_parameter("outT", [D, Q], F32, isOutput=True)

    # dram views with row-tiles on partitions
    qT_t = qT.rearrange("(t p) l -> p t l", p=128)  # [128, 4, Q]
    kT_t = kT.rearrange("(t p) l -> p t l", p=128)  # [128, 4, L]
    vT_t = vT.rearrange("(t p) l -> p t l", p=128)
    WqT_t = WqT.rearrange("(t p) d -> p t d", p=128)  # [128, 4, 512]
    WkT_t = WkT.rearrange("(t p) d -> p t d", p=128)
    WvT_t = WvT.rearrange("(t p) d -> p t d", p=128)
    WoT_t = WoT.rearrange("(t p) d -> p t d", p=128)
    outT_t = outT.rearrange("(t p) l -> p t l", p=128)

    KT = L // 128  # 16 k-tiles
    QT = Q // 128  # 8 q-tiles

    with TileContext(nc) as tc:
        with (
            tc.tile_pool(name="persist", bufs=1) as persist,
            tc.tile_pool(name="weights", bufs=1) as wpool,
            tc.tile_pool(name="stream", bufs=2) as stream,
            tc.tile_pool(name="pslab", bufs=2) as pslab,
            tc.tile_pool(name="outp", bufs=2) as outp,
            tc.tile_pool(name="ps_big", bufs=3, space="PSUM") as ps_big,
            tc.tile_pool(name="ps_small", bufs=2, space="PSUM") as ps_small,
        ):
            # ---- constants / weights ----
            # Load order matters: what gates the first matmuls goes first.
            w_q = wpool.tile([128, 4, D], BF16, tag="wq")
            w_k = wpool.tile([128, 4, D], BF16, tag="wk")
            w_v = wpool.tile([128, 4, D], BF16, tag="wv")
            w_o = persist.tile([128, 4, D], BF16, tag="wo")
            kc_all = wpool.tile([128, 4, L], BF16, tag="kcall")
            b_q = persist.tile([128, 4], F32, tag="bq")
            b_k = persist.tile([128, 4], F32, tag="bk")
            b_o = persist.tile([128, 4], F32, tag="bo")
            b_v = persist.tile([1, D + 128], F32R, tag="bv")
            nc.sync.dma_start(out=w_q, in_=WqT_t)
            nc.sync.dma_start(out=b_q, in_=bqt[:, :])
            nc.sync.dma_start(out=w_k, in_=WkT_t)
            nc.sync.dma_start(out=b_k, in_=bkt[:, :])
            nc.sync.dma_start(out=kc_all, in_=kT_t)
            ones_row = b_v[0:1, D : D + 128]

            # ---- persistent activations ----
            # khT / qhT: 4 tiles of [128, L] fp32; partition = feature (2 heads/tile)
            khT = [persist.tile([128, L], BF16, tag=f"khT{t}", name=f"khT{t}") for t in range(4)]
            qhT = [persist.tile([128, Q], BF16, tag=f"qhT{t}", name=f"qhT{t}") for t in range(4)]
            # vh natural, bf16, augmented ones col: [128, kt, h, 65]
            vh = persist.tile([128, KT, H, HD + 1], BF16, tag="vh")
            nc.vector.memset(vh[:, :, :, HD : HD + 1], 1.0)
            # attention output transposed: 4 tiles [128, Q] fp32
            oVT = [persist.tile([128, Q], BF16, tag=f"oVT{t}", name=f"oVT{t}") for t in range(4)]

            # ---- projections ----
            LC = 4  # l-chunks of 512 for k/v

            def emit_kh(dt):
                for lc in range(LC):
                    ps = ps_small.tile([128, 512], F32, tag="mm")
                    for kt in range(4):
                        nc.tensor.matmul(
                            ps,
                            lhsT=w_k[:, kt, dt * 128 : (dt + 1) * 128],
                            rhs=kc_all[:, kt, lc * 512 : (lc + 1) * 512],
                            start=(kt == 0),
                            stop=(kt == 3),
                        )
                    nc.vector.tensor_scalar(
                        out=khT[dt][:, lc * 512 : (lc + 1) * 512],
                        in0=ps,
                        scalar1=b_k[:, dt : dt + 1],
                        scalar2=0.0,
                        op0=mybir.AluOpType.add,
                        op1=mybir.AluOpType.max,
                    )

            # q projection first (it gates scores of head 0)
            for lc in range(2):  # q chunks (Q=1024)
                qc = stream.tile([128, 4, 512], BF16, tag="chunk")
                nc.sync.dma_start(out=qc, in_=qT_t[:, :, lc * 512 : (lc + 1) * 512])
                for dt in range(4):
                    ps = ps_small.tile([128, 512], F32, tag="mm")
                    for kt in range(4):
                        nc.tensor.matmul(
                            ps,
                            lhsT=w_q[:, kt, dt * 128 : (dt + 1) * 128],
                            rhs=qc[:, kt, :],
                            start=(kt == 0),
                            stop=(kt == 3),
                        )
                    nc.vector.tensor_scalar(
                        out=qhT[dt][:, lc * 512 : (lc + 1) * 512],
                        in0=ps,
                        scalar1=b_q[:, dt : dt + 1],
                        scalar2=0.0,
                        op0=mybir.AluOpType.add,
                        op1=mybir.AluOpType.max,
                    )

            emit_kh(0)  # heads 0/1 ready -> scores(0) can start

            # ---- attention per head ----
            # Normalization runs one (h,qch) iteration behind the attnV
            # matmul groups so the PE stream never waits on DVE.
            def emit_norm(pend):
                rc, oU, tht, tho, qsl = pend
                ps_b = ps_small.tile([HD, 512], F32, tag="mm")
                nc.tensor.matmul(
                    ps_b,
                    lhsT=ones_row[0:1, 0:HD],
                    rhs=rc,
                    start=True,
                    stop=True,
                )
                recipB = outp.tile([HD, 512], F32, tag="recipB", bufs=1)
                nc.vector.tensor_copy(out=recipB, in_=ps_b)
                nc.vector.tensor_tensor(
                    out=oVT[tht][tho : tho + 64, qsl],
                    in0=oU,
                    in1=recipB,
                    op=mybir.AluOpType.mult,
                )

            P_tiles = {}
            state = {"pend": None}

            def emit_scores(h, kts):
                ht, ho = h // 2, (h % 2) * 64
                P = P_tiles[h]
                for kt in kts:
                    ps_s = ps_big.tile([128, Q], F32, tag="scores")
                    for qh2 in range(2):
                        nc.tensor.matmul(
                            ps_s[:, qh2 * 512 : (qh2 + 1) * 512],
                            lhsT=khT[ht][ho : ho + 64, kt * 128 : (kt + 1) * 128],
                            rhs=qhT[ht][ho : ho + 64, qh2 * 512 : (qh2 + 1) * 512],
                            start=True,
                            stop=True,
                        )
                    nc.scalar.activation(
                        out=P[:, kt, :],
                        in_=ps_s,
                        func=mybir.ActivationFunctionType.Exp,
                        scale=SCALE,
                    )

            def emit_attnv(h, qch):
                ht, ho = h // 2, (h % 2) * 64
                qsl = slice(qch * 512, (qch + 1) * 512)
                P = P_tiles[h]
                ps_av = ps_small.tile([HD + 1, 512], F32, tag="mm")
                for kt in range(KT):
                    nc.tensor.matmul(
                        ps_av,
                        lhsT=vh[:, kt, h, :],
                        rhs=P[:, kt, qsl],
                        start=(kt == 0),
                        stop=(kt == KT - 1),
                    )
                oU = outp.tile([HD, 512], F32, tag="oU")
                nc.vector.tensor_copy(out=oU, in_=ps_av[0:HD, :])
                rc = outp.tile([1, 512], F32R, tag="fout")
                with nc.allow_low_precision(reason="f32r recip bcast"):
                    nc.vector.reciprocal(out=rc, in_=ps_av[HD : HD + 1, :])
                if state["pend"] is not None:
                    emit_norm(state["pend"])
                state["pend"] = (rc, oU, ht, ho, qsl)

            # software pipeline: scores(h+1) halves interleave with attnV(h)
            # chunks so ACT (exp) always has fresh score tiles to chew on.
            P_tiles[0] = pslab.tile([128, KT, Q], BF16, tag="P", name="P0")
            emit_scores(0, range(KT))

            # remaining projections overlap the head-0 exp stream
            nc.sync.dma_start(out=w_v, in_=WvT_t)
            nc.sync.dma_start(out=b_v, in_=bvr[:, :])
            nc.sync.dma_start(out=w_o, in_=WoT_t)
            nc.sync.dma_start(out=b_o, in_=bot[:, :])
            for dt in range(1, 4):
                emit_kh(dt)
            for lc in range(LC):
                vc = stream.tile([128, 4, 512], BF16, tag="chunk")
                nc.sync.dma_start(out=vc, in_=vT_t[:, :, lc * 512 : (lc + 1) * 512])
                for lt4 in range(4):
                    lt = lc * 4 + lt4  # global l-tile (= k-tile index)
                    ps = ps_small.tile([128, 512], F32, tag="mm")
                    for kt in range(4):
                        nc.tensor.matmul(
                            ps,
                            lhsT=vc[:, kt, lt4 * 128 : (lt4 + 1) * 128],
                            rhs=w_v[:, kt, :],
                            start=(kt == 0),
                            stop=False,
                        )
                    nc.tensor.matmul(
                        ps,
                        lhsT=ones_row,
                        rhs=b_v[0:1, 0:D],
                        start=False,
                        stop=True,
                    )
                    nc.vector.tensor_scalar(
                        out=vh[:, lt, :, 0:HD],
                        in0=ps.rearrange("p (h d) -> p h d", h=H),
                        scalar1=0.0,
                        scalar2=None,
                        op0=mybir.AluOpType.max,
                    )

            for h in range(H):
                if h + 1 < H:
                    P_tiles[h + 1] = pslab.tile(
                        [128, KT, Q], BF16, tag="P", name=f"P{h + 1}"
                    )
                    emit_scores(h + 1, range(0, KT // 2))
                emit_attnv(h, 0)
                if h + 1 < H:
                    emit_scores(h + 1, range(KT // 2, KT))
                emit_attnv(h, 1)
                del P_tiles[h]
            emit_norm(state["pend"])

            # ---- output projection ----
            for qc2 in range(2):
                for dt in range(4):
                    ps = ps_small.tile([128, 512], F32, tag="mm")
                    for ktile in range(4):
                        nc.tensor.matmul(
                            ps,
                            lhsT=w_o[:, ktile, dt * 128 : (dt + 1) * 128],
                            rhs=oVT[ktile][:, qc2 * 512 : (qc2 + 1) * 512],
                            start=(ktile == 0),
                            stop=(ktile == 3),
                        )
                    fo = outp.tile([128, 512], F32, tag="fout")
                    nc.vector.tensor_scalar(
                        out=fo,
                        in0=ps,
                        scalar1=b_o[:, dt : dt + 1],
                        scalar2=0.0,
                        op0=mybir.AluOpType.add,
                        op1=mybir.AluOpType.max,
                    )
                    nc.sync.dma_start(
                        out=outT_t[:, dt, qc2 * 512 : (qc2 + 1) * 512], in_=fo
                    )

    if waitfix:
        _split_sem_waits(nc, max_waits=1)
    return nc


_NC = None


def _get_nc():
    global _NC
    if _NC is None:
        _NC = build_nc()
    return _NC


def kernel(**inputs):
    q = np.asarray(inputs["q"], np.float32)
    k = np.asarray(inputs["k"], np.float32)
    v = np.asarray(inputs["v"], np.float32)
    Wq = np.asarray(inputs["Wq"], np.float32)
    Wk = np.asarray(inputs["Wk"], np.float32)
    Wv = np.asarray(inputs["Wv"], np.float32)
    Wo = np.asarray(inputs["Wo"], np.float32)
    bq = np.asarray(inputs["bq"], np.float32)
    bk = np.asarray(inputs["bk"], np.float32)
    bv = np.asarray(inputs["bv"], np.float32)
    bo = np.asarray(inputs["bo"], np.float32)

    nc = _get_nc()

    import ml_dtypes

    bf16 = ml_dtypes.bfloat16
    WqT = np.ascontiguousarray(Wq.T).astype(bf16)
    WkT = np.ascontiguousarray(Wk.T).astype(bf16)
    WvT = np.ascontiguousarray(Wv.T).astype(bf16)
    WoT = np.ascontiguousarray(Wo.T).astype(bf16)
    bqt = np.ascontiguousarray(bq.reshape(4, 128).T)
    bkt = np.ascontiguousarray(bk.reshape(4, 128).T)
    bot = np.ascontiguousarray(bo.reshape(4, 128).T)
    bvr = np.empty((1, D + 128), np.float32)
    bvr[0, :D] = bv
    bvr[0, D:] = 1.0
    idn = np.eye(128, dtype=np.float32)

    qTs = [np.ascontiguousarray(q[b].T).astype(bf16) for b in range(B)]
    kTs = [np.ascontiguousarray(k[b].T).astype(bf16) for b in range(B)]
    vTs = [np.ascontiguousarray(v[b].T).astype(bf16) for b in range(B)]

    in_maps = []
    for c in range(N_CORES):
        b, lh = c // 2, c % 2
        in_maps.append(
            {
                "qT": np.ascontiguousarray(qTs[b][:, lh * Q : (lh + 1) * Q]),
                "kT": kTs[b],
                "vT": vTs[b],
                "WqT": WqT,
                "WkT": WkT,
                "WvT": WvT,
                "WoT": WoT,
                "bqt": bqt,
                "bkt": bkt,
                "bot": bot,
                "bvr": bvr,
                "idn": idn,
            }
        )

    res = run_bass_kernel_spmd(nc, in_maps, core_ids=list(range(N_CORES)))

    out = np.empty((B, L, D), np.float32)
    for c in range(N_CORES):
        b, lh = c // 2, c % 2
        out[b, lh * Q : (lh + 1) * Q, :] = res.results[c]["outT"].T
    return out


# revision 35
# speedup vs baseline: 1.8125x; 1.1637x over previous
"""MultiHeadAttention Trainium2 Bass kernel.

Reference math (B=4, L=2048, D=512, H=8, HD=64):
    qh = relu(q @ Wq.T + bq) ; kh = relu(k @ Wk.T + bk) ; vh = relu(v @ Wv.T + bv)
    scores = (qh_heads @ kh_heads.T) / sqrt(512)
    out = softmax(scores) @ vh_heads   (per head)
    out = relu(concat_heads(out) @ Wo.T + bo)
rel_k_table / rel_v_table are dead inputs.

Sharding: 8 cores = (batch b in 0..3) x (query-half lh in 0..1).
Each core computes the full output rows [b, lh*1024:(lh+1)*1024, :].
k/v projections are duplicated between the 2 cores of a batch (cheap) so
there is NO cross-core communication; host only transposes/concats.

On-device layout is fully transposed ("feature dim on partitions"):
  inputs uploaded as qT [512,1024], kT [512,2048], vT [512,2048] (host transpose)
  qhT/khT: [64*H rows, L] computed as  W.T-tiles (lhsT) @ xT (rhs)   [f32r]
  vh:      natural [l, do] via xT-tiles (lhsT) @ WvT (rhs), bias via ones-row
           matmul, relu on DVE -> bf16, augmented with a ones column (65 wide)
  scores_T[k, q] per head = khT-tile (lhsT, K=dh=64) @ qhT (rhs)     [f32r]
  P = exp(scores * 1/sqrt(512)) on ACT (N=1024 per instr) -> bf16
  attnV natural: P_T-tile (lhsT) @ vh_aug (rhs, N=65) -> [q,64|denom] [bf16]
  normalize rows by 1/denom (per-partition scalar on DVE), PE-transpose
  final_T = WoT-tiles (lhsT) @ oVT (rhs) + relu/bias                 [f32r]
  output stored transposed [512, 1024]; host transposes back.
"""

import sys

sys.path.insert(0, "/opt/trn_rl_repo")

import numpy as np

import concourse.bass as bass
import concourse.mybir as mybir
from concourse.tile import TileContext
from concourse.bass_utils import run_bass_kernel_spmd

B, L, D, H = 4, 2048, 512, 8
HD = D // H  # 64
Q = L // 2  # queries per core (1024)
SCALE = 1.0 / float(np.sqrt(D))

F32 = mybir.dt.float32
F32R = mybir.dt.float32r
BF16 = mybir.dt.bfloat16

N_CORES = 8


def _split_sem_waits(nc, max_waits=1):
    """walrus in this container only accepts one sem-wait per instruction;
    split extra waits onto preceding NoOps on the same engine."""
    ctr = [0]

    def mknop(engine, waits):
        ctr[0] += 1
        n = mybir.InstNoOp(name=f"I-waitfix-{ctr[0]}", ins=[], outs=[])
        n.engine = engine
        n.sync_info = mybir.SyncInfo(on_wait=list(waits), on_update=[])
        return n

    for fn in nc.m.functions:
        for bb in fn.blocks:
            changed = False
            new = []
            for inst in bb.instructions:
                si = inst.sync_info
                if si is not None and si.on_wait and len(si.on_wait) > max_waits:
                    waits = list(si.on_wait)
                    extra, keep = waits[:-max_waits], waits[-max_waits:]
                    for i in range(0, len(extra), max_waits):
                        new.append(mknop(inst.engine, extra[i : i + max_waits]))
                    inst.sync_info = mybir.SyncInfo(
                        on_wait=keep, on_update=list(si.on_update)
                    )
                    changed = True
                new.append(inst)
            if changed:
                bb.instructions = new


def _raise_sbuf_limit():
    try:
        from concourse import tile_utils

        if getattr(tile_utils, "max_sbuf_usage", 0) < 206 * 1024:
            tile_utils.max_sbuf_usage = 206 * 1024
    except Exception:
        pass


def build_nc(waitfix=True):
    _raise_sbuf_limit()
    nc = bass.Bass()

    qT = nc.declare_dram_parameter("qT", [D, Q], BF16, isOutput=False)
    kT = nc.declare_dram_parameter("kT", [D, L], BF16, isOutput=False)
    vT = nc.declare_dram_parameter("vT", [D, L], BF16, isOutput=False)
    WqT = nc.declare_dram_parameter("WqT", [D, D], BF16, isOutput=False)
    WkT = nc.declare_dram_parameter("WkT", [D, D], BF16, isOutput=False)
    WvT = nc.declare_dram_parameter("WvT", [D, D], BF16, isOutput=False)
    WoT = nc.declare_dram_parameter("WoT", [D, D], BF16, isOutput=False)
    # biases pre-tiled on host: [128, 4] column t = bias slice for do-tile t
    bqt = nc.declare_dram_parameter("bqt", [128, 4], F32, isOutput=False)
    bkt = nc.declare_dram_parameter("bkt", [128, 4], F32, isOutput=False)
    bot = nc.declare_dram_parameter("bot", [128, 4], F32, isOutput=False)
    bvr = nc.declare_dram_parameter("bvr", [1, D + 128], F32R, isOutput=False)
    idn = nc.declare_dram_parameter("idn", [128, 128], F32, isOutput=False)
    outT = nc.declare_dram_parameter("outT", [D, Q], F32, isOutput=True)

    # dram views with row-tiles on partitions
    qT_t = qT.rearrange("(t p) l -> p t l", p=128)  # [128, 4, Q]
    kT_t = kT.rearrange("(t p) l -> p t l", p=128)  # [128, 4, L]
    vT_t = vT.rearrange("(t p) l -> p t l", p=128)
    WqT_t = WqT.rearrange("(t p) d -> p t d", p=128)  # [128, 4, 512]
    WkT_t = WkT.rearrange("(t p) d -> p t d", p=128)
    WvT_t = WvT.rearrange("(t p) d -> p t d", p=128)
    WoT_t = WoT.rearrange("(t p) d -> p t d", p=128)
    outT_t = outT.rearrange("(t p) l -> p t l", p=128)

    KT = L // 128  # 16 k-tiles
    QT = Q // 128  # 8 q-tiles

    with TileContext(nc) as tc:
        with (
            tc.tile_pool(name="persist", bufs=1) as persist,
            tc.tile_pool(name="weights", bufs=1) as wpool,
            tc.tile_pool(name="stream", bufs=2) as stream,
            tc.tile_pool(name="pslab", bufs=2) as pslab,
            tc.tile_pool(name="outp", bufs=2) as outp,
            tc.tile_pool(name="ps_big", bufs=3, space="PSUM") as ps_big,
            tc.tile_pool(name="ps_small", bufs=2, space="PSUM") as ps_small,
        ):
            # ---- constants / weights ----
            # Load order matters: what gates the first matmuls goes first.
            w_q = wpool.tile([128, 4, D], BF16, tag="wq")
            w_k = wpool.tile([128, 4, D], BF16, tag="wk")
            w_v = wpool.tile([128, 4, D], BF16, tag="wv")
            w_o = persist.tile([128, 4, D], BF16, tag="wo")
            kc_all = wpool.tile([128, 4, L], BF16, tag="kcall")
            b_q = persist.tile([128, 4], F32, tag="bq")
            b_k = persist.tile([128, 4], F32, tag="bk")
            b_o = persist.tile([128, 4], F32, tag="bo")
            b_v = persist.tile([1, D + 128], F32R, tag="bv")
            nc.sync.dma_start(out=w_q, in_=WqT_t)
            nc.sync.dma_start(out=b_q, in_=bqt[:, :])
            ones_row = b_v[0:1, D : D + 128]

            # ---- persistent activations ----
            # khT / qhT: 4 tiles of [128, L] fp32; partition = feature (2 heads/tile)
            khT = [persist.tile([128, L], BF16, tag=f"khT{t}", name=f"khT{t}") for t in range(4)]
            qhT = [persist.tile([128, Q], BF16, tag=f"qhT{t}", name=f"qhT{t}") for t in range(4)]
            # vh natural, bf16, augmented ones col: [128, kt, h, 65]
            vh = persist.tile([128, KT, H, HD + 1], BF16, tag="vh")
            nc.vector.memset(vh[:, :, :, HD : HD + 1], 1.0)
            # attention output transposed: 4 tiles [128, Q] fp32
            oVT = [persist.tile([128, Q], BF16, tag=f"oVT{t}", name=f"oVT{t}") for t in range(4)]

            # ---- projections ----
            LC = 4  # l-chunks of 512 for k/v

            def emit_kh(dt):
                for lc in range(LC):
                    ps = ps_small.tile([128, 512], F32, tag="mm")
                    for kt in range(4):
                        nc.tensor.matmul(
                            ps,
                            lhsT=w_k[:, kt, dt * 128 : (dt + 1) * 128],
                            rhs=kc_all[:, kt, lc * 512 : (lc + 1) * 512],
                            start=(kt == 0),
                            stop=(kt == 3),
                        )
                    nc.vector.tensor_scalar(
                        out=khT[dt][:, lc * 512 : (lc + 1) * 512],
                        in0=ps,
                        scalar1=b_k[:, dt : dt + 1],
                        scalar2=0.0,
                        op0=mybir.AluOpType.add,
                        op1=mybir.AluOpType.max,
                    )

            # q projection first (it gates scores of head 0)
            for lc in range(2):  # q chunks (Q=1024)
                qc = stream.tile([128, 4, 512], BF16, tag="chunk")
                nc.sync.dma_start(out=qc, in_=qT_t[:, :, lc * 512 : (lc + 1) * 512])
                if lc == 1:
                    nc.sync.dma_start(out=w_k, in_=WkT_t)
                    nc.sync.dma_start(out=b_k, in_=bkt[:, :])
                    nc.sync.dma_start(out=kc_all, in_=kT_t)
                for dt in range(4):
                    ps = ps_small.tile([128, 512], F32, tag="mm")
                    for kt in range(4):
                        nc.tensor.matmul(
                            ps,
                            lhsT=w_q[:, kt, dt * 128 : (dt + 1) * 128],
                            rhs=qc[:, kt, :],
                            start=(kt == 0),
                            stop=(kt == 3),
                        )
                    nc.vector.tensor_scalar(
                        out=qhT[dt][:, lc * 512 : (lc + 1) * 512],
                        in0=ps,
                        scalar1=b_q[:, dt : dt + 1],
                        scalar2=0.0,
                        op0=mybir.AluOpType.add,
                        op1=mybir.AluOpType.max,
                    )

            emit_kh(0)  # heads 0/1 ready -> scores(0) can start

            # ---- attention per head ----
            # Normalization runs one (h,qch) iteration behind the attnV
            # matmul groups so the PE stream never waits on DVE.
            def emit_norm(pend):
                rc, oU, tht, tho, qsl = pend
                ps_b = ps_small.tile([HD, 512], F32, tag="mm")
                nc.tensor.matmul(
                    ps_b,
                    lhsT=ones_row[0:1, 0:HD],
                    rhs=rc,
                    start=True,
                    stop=True,
                )
                recipB = outp.tile([HD, 512], F32, tag="recipB", bufs=1)
                nc.vector.tensor_copy(out=recipB, in_=ps_b)
                nc.vector.tensor_tensor(
                    out=oVT[tht][tho : tho + 64, qsl],
                    in0=oU,
                    in1=recipB,
                    op=mybir.AluOpType.mult,
                )

            P_tiles = {}
            state = {"pend": None}

            def emit_scores(h, kts):
                ht, ho = h // 2, (h % 2) * 64
                P = P_tiles[h]
                for kt in kts:
                    ps_s = ps_big.tile([128, Q], F32, tag="scores")
                    for qh2 in range(2):
                        nc.tensor.matmul(
                            ps_s[:, qh2 * 512 : (qh2 + 1) * 512],
                            lhsT=khT[ht][ho : ho + 64, kt * 128 : (kt + 1) * 128],
                            rhs=qhT[ht][ho : ho + 64, qh2 * 512 : (qh2 + 1) * 512],
                            start=True,
                            stop=True,
                        )
                    nc.scalar.activation(
                        out=P[:, kt, :],
                        in_=ps_s,
                        func=mybir.ActivationFunctionType.Exp,
                        scale=SCALE,
                    )

            def emit_attnv(h, qch):
                ht, ho = h // 2, (h % 2) * 64
                qsl = slice(qch * 512, (qch + 1) * 512)
                P = P_tiles[h]
                ps_av = ps_small.tile([HD + 1, 512], F32, tag="mm")
                for kt in range(KT):
                    nc.tensor.matmul(
                        ps_av,
                        lhsT=vh[:, kt, h, :],
                        rhs=P[:, kt, qsl],
                        start=(kt == 0),
                        stop=(kt == KT - 1),
                    )
                oU = outp.tile([HD, 512], F32, tag="oU")
                nc.vector.tensor_copy(out=oU, in_=ps_av[0:HD, :])
                # norm of the PREVIOUS chunk goes on the DVE queue ahead of
                # this chunk's slow reciprocal, so the bcast-psum slot frees
                # quickly and PE never stalls on it.
                if state["pend"] is not None:
                    emit_norm(state["pend"])
                rc = outp.tile([1, 512], F32R, tag="fout")
                with nc.allow_low_precision(reason="f32r recip bcast"):
                    nc.vector.reciprocal(out=rc, in_=ps_av[HD : HD + 1, :])
                state["pend"] = (rc, oU, ht, ho, qsl)

            def emit_vh(lc):
                vc = stream.tile([128, 4, 512], BF16, tag="chunk", name=f"vc{lc}")
                nc.sync.dma_start(out=vc, in_=vT_t[:, :, lc * 512 : (lc + 1) * 512])
                for lt4 in range(4):
                    lt = lc * 4 + lt4  # global l-tile (= k-tile index)
                    ps = ps_small.tile([128, 512], F32, tag="mm")
                    for kt in range(4):
                        nc.tensor.matmul(
                            ps,
                            lhsT=vc[:, kt, lt4 * 128 : (lt4 + 1) * 128],
                            rhs=w_v[:, kt, :],
                            start=(kt == 0),
                            stop=False,
                        )
                    nc.tensor.matmul(
                        ps,
                        lhsT=ones_row,
                        rhs=b_v[0:1, 0:D],
                        start=False,
                        stop=True,
                    )
                    nc.vector.tensor_scalar(
                        out=vh[:, lt, :, 0:HD],
                        in0=ps.rearrange("p (h d) -> p h d", h=H),
                        scalar1=0.0,
                        scalar2=None,
                        op0=mybir.AluOpType.max,
                    )

            # software pipeline: scores(h+1) halves interleave with attnV(h)
            # chunks so ACT (exp) always has fresh score tiles to chew on.
            # Remaining projection work (kh dt 1-3, all of vh) is woven into
            # the first iterations, overlapping the head-0/1 exp streams.
            P_tiles[0] = pslab.tile([128, KT, Q], BF16, tag="P", name="P0")
            emit_scores(0, range(KT))
            nc.sync.dma_start(out=w_v, in_=WvT_t)
            nc.sync.dma_start(out=b_v, in_=bvr[:, :])
            nc.sync.dma_start(out=w_o, in_=WoT_t)
            nc.sync.dma_start(out=b_o, in_=bot[:, :])
            emit_kh(1)

            for h in range(H):
                if h + 1 < H:
                    P_tiles[h + 1] = pslab.tile(
                        [128, KT, Q], BF16, tag="P", name=f"P{h + 1}"
                    )
                    emit_scores(h + 1, range(0, KT // 2))
                if h == 0:
                    for lc in range(LC):
                        emit_vh(lc)
                emit_attnv(h, 0)
                if h + 1 < H:
                    emit_scores(h + 1, range(KT // 2, KT))
                emit_attnv(h, 1)
                if h == 0:
                    emit_kh(2)
                elif h == 1:
                    emit_kh(3)
                del P_tiles[h]
            emit_norm(state["pend"])

            # ---- output projection ----
            for qc2 in range(2):
                for dt in range(4):
                    ps = ps_small.tile([128, 512], F32, tag="mm")
                    for ktile in range(4):
                        nc.tensor.matmul(
                            ps,
                            lhsT=w_o[:, ktile, dt * 128 : (dt + 1) * 128],
                            rhs=oVT[ktile][:, qc2 * 512 : (qc2 + 1) * 512],
                            start=(ktile == 0),
                            stop=(ktile == 3),
                        )
                    fo = outp.tile([128, 512], F32, tag="fout")
                    nc.vector.tensor_scalar(
                        out=fo,
                        in0=ps,
                        scalar1=b_o[:, dt : dt + 1],
                        scalar2=0.0,
                        op0=mybir.AluOpType.add,
                        op1=mybir.AluOpType.max,
                    )
                    nc.sync.dma_start(
                        out=outT_t[:, dt, qc2 * 512 : (qc2 + 1) * 512], in_=fo
                    )

    if waitfix:
        _split_sem_waits(nc, max_waits=1)
    return nc


_NC = None


def _get_nc():
    global _NC
    if _NC is None:
        _NC = build_nc()
    return _NC


def kernel(**inputs):
    q = np.asarray(inputs["q"], np.float32)
    k = np.asarray(inputs["k"], np.float32)
    v = np.asarray(inputs["v"], np.float32)
    Wq = np.asarray(inputs["Wq"], np.float32)
    Wk = np.asarray(inputs["Wk"], np.float32)
    Wv = np.asarray(inputs["Wv"], np.float32)
    Wo = np.asarray(inputs["Wo"], np.float32)
    bq = np.asarray(inputs["bq"], np.float32)
    bk = np.asarray(inputs["bk"], np.float32)
    bv = np.asarray(inputs["bv"], np.float32)
    bo = np.asarray(inputs["bo"], np.float32)

    nc = _get_nc()

    import ml_dtypes

    bf16 = ml_dtypes.bfloat16
    WqT = np.ascontiguousarray(Wq.T).astype(bf16)
    WkT = np.ascontiguousarray(Wk.T).astype(bf16)
    WvT = np.ascontiguousarray(Wv.T).astype(bf16)
    WoT = np.ascontiguousarray(Wo.T).astype(bf16)
    bqt = np.ascontiguousarray(bq.reshape(4, 128).T)
    bkt = np.ascontiguousarray(bk.reshape(4, 128).T)
    bot = np.ascontiguousarray(bo.reshape(4, 128).T)
    bvr = np.empty((1, D + 128), np.float32)
    bvr[0, :D] = bv
    bvr[0, D:] = 1.0
    idn = np.eye(128, dtype=np.float32)

    qTs = [np.ascontiguousarray(q[b].T).astype(bf16) for b in range(B)]
    kTs = [np.ascontiguousarray(k[b].T).astype(bf16) for b in range(B)]
    vTs = [np.ascontiguousarray(v[b].T).astype(bf16) for b in range(B)]

    in_maps = []
    for c in range(N_CORES):
        b, lh = c // 2, c % 2
        in_maps.append(
            {
                "qT": np.ascontiguousarray(qTs[b][:, lh * Q : (lh + 1) * Q]),
                "kT": kTs[b],
                "vT": vTs[b],
                "WqT": WqT,
                "WkT": WkT,
                "WvT": WvT,
                "WoT": WoT,
                "bqt": bqt,
                "bkt": bkt,
                "bot": bot,
                "bvr": bvr,
                "idn": idn,
            }
        )

    res = run_bass_kernel_spmd(nc, in_maps, core_ids=list(range(N_CORES)))

    out = np.empty((B, L, D), np.float32)
    for c in range(N_CORES):
        b, lh = c // 2, c % 2
        out[b, lh * Q : (lh + 1) * Q, :] = res.results[c]["outT"].T
    return out
